# revision 15
# baseline (speedup 1.0000x reference)
"""Trainium2 Bass kernel for nn_EngramAttention (causal MHA block).

Computes: qkv = x @ Wqkv + bqkv; causal 16-head attention; out @ Wout + bout.
Shapes: x [2, 2048, 1024], Wqkv [1024, 3072], Wout [1024, 1024].

Sharding (8 NeuronCores, tensor-parallel by heads):
  - core c owns heads {2c, 2c+1} (128 feature columns of each of Q/K/V).
  - Every core reads all tokens (x fed pre-transposed, feature-major, bf16).
  - Per batch b: QKV projection, causal attention in scoresT layout (keys on
    partitions; softmax denominator via an appended ones-row in the PV
    matmul).  The two head-halves' scores matmuls are emitted pairwise so
    they run CONCURRENTLY on disjoint PE row groups (stationary kT at
    partitions 0-63 vs 64-127).  Attention outputs ship UN-normalized with
    the denominator row ([65, 256] blocks) through per-(batch, head-half)
    AllToAlls; each core ends with all 1024 features for its 256-token slice
    of each batch, normalizes post-A2A (reciprocal + selection-matrix
    broadcast matmuls) and runs the output projection locally.
  - batch-1 QKV weaves under batch-0 attention; batch-0 normalization +
    output projection weave under batch-1 attention; the last two A2As are
    staggered (h1 before h0's final j=3 stage) so the tail overlaps
    collective latency with the h1-half output projection.  Post-collective
    work is pinned behind independent stage work with nosync dep edges so
    the scheduler cannot hoist it into the engine streams early (which
    would stall the machine on collective latency).

All matmuls run in bf16 (fp32 accumulation in PSUM).
"""

import os
import sys

for _p in ("/opt/trn_rl_repo", "/root/.axon_site/_ro/trn_rl_repo"):
    if os.path.isdir(_p) and _p not in sys.path:
        sys.path.insert(0, _p)

import ml_dtypes
import numpy as np

import concourse.bass as bass
import concourse.mybir as mybir
import concourse.tile as tile
from concourse.bass_utils import run_bass_kernel_spmd
from concourse.masks import make_identity
from concourse.tile_rust import add_dep_helper
from concourse.vector_clock import ScopedClock

BF16 = mybir.dt.bfloat16
F32 = mybir.dt.float32
NPBF16 = ml_dtypes.bfloat16

NCORES = 8
D = 1024          # hidden
NTOK = 4096       # B*T
T = 2048
B = 2
FEAT = 128        # per-core head features (2 heads x 64)
TOKC = 256        # per-core per-batch token slice in the output projection
SCALE = 0.125     # 1/sqrt(64)

# module-level handles for optional tracing by test harnesses
TRACE = False
TRACE_KWARGS = {}
LAST_RESULT = None


class _SplitDrainTileContext(tile.TileContext):
    """TileContext whose tail drain splits semaphore waits one-per-instruction.

    The walrus build in this container rejects >N sync waits on a single
    Drain ("Too many sync wait commands"), so emit a chain of drains, each
    carrying a single wait, instead of one drain carrying all of them.
    """

    def _drain_and_barrier(self, tick_clock, wait_clock):
        nc = self.nc
        drain_inst = nc.sync.drain()
        wait_clock.add_sem_waits(
            drain_inst.ins, ScopedClock({None: tick_clock.global_clock})
        )
        si = drain_inst.ins.sync_info
        if si is not None and si.on_wait and len(si.on_wait) > 1:
            waits = list(si.on_wait)
            drain_inst.ins.sync_info = mybir.SyncInfo(
                on_wait=waits[:1], on_update=list(si.on_update or [])
            )
            for w in waits[1:]:
                d2 = nc.sync.drain()
                si2 = d2.ins.sync_info
                upd = list(si2.on_update or []) if si2 is not None else []
                d2.ins.sync_info = mybir.SyncInfo(on_wait=[w], on_update=upd)

        nc.all_engine_barrier()
        assert self.sems is not None
        popped = nc._tile_sem_poison_stack.pop()
        assert popped is self._sem_poison
        nc.clear_and_free_semaphores(list(self.sems.allocated().values()))
        nc.all_engine_barrier()


def _split_excess_waits(nc, auxes, max_waits=1):
    """Walrus in this container rejects instructions carrying more than a
    couple of semaphore waits ("Too many sync wait commands").  Move excess
    waits onto EventSemaphore carrier instructions inserted just before the
    offending instruction on the same engine (same-engine FIFO order makes
    this semantically identical).

    DMA instructions execute on the DMA-queue processors, asynchronously
    from the issuing engine's stream, so an engine-side carrier alone would
    NOT order them.  For those, the carrier chain additionally increments an
    auxiliary semaphore and the DMA itself waits on it — the DMA then
    carries exactly one wait."""
    n = 0
    aux_counts = {}
    dma_ops = ("DMACopy", "DMATranspose", "TriggeredCopy")

    def _carrier(engine, wait_grp):
        nonlocal n
        ev = mybir.InstEventSemaphore(
            name=f"wsplit-{n}",
            engine=engine,
            ins=[],
            outs=[],
            sync_info=mybir.SyncInfo(on_wait=list(wait_grp), on_update=[]),
        )
        n += 1
        nc.register_instruction(ev, overwrite=True)
        return ev

    for fn in nc.m.functions:
        for blk in fn.blocks:
            out = []
            for ins in blk.instructions:
                si = ins.sync_info
                waits = list(si.on_wait) if (si is not None and si.on_wait) else []
                if len(waits) > max_waits:
                    if ins.opcode in dma_ops:
                        # one aux chain PER ENGINE: a carrier stuck behind a
                        # collective must not transitively stall split DMAs
                        # on the sync ring (global-chain HOL)
                        aux = auxes[ins.engine]
                        for w in waits:
                            out.append(_carrier(ins.engine, [w]))
                        cnt = aux_counts.get(ins.engine, 0) + 1
                        aux_counts[ins.engine] = cnt
                        bass.BassInstruction(out[-1]).then_inc(aux, 1)
                        ins.sync_info = mybir.SyncInfo(
                            on_wait=[], on_update=list(si.on_update or [])
                        )
                        bass.BassInstruction(ins).wait_op(
                            aux, cnt, "sem-ge"
                        )
                    else:
                        extra, keep = waits[:-max_waits], waits[-max_waits:]
                        for i in range(0, len(extra), max_waits):
                            out.append(_carrier(ins.engine, extra[i : i + max_waits]))
                        ins.sync_info = mybir.SyncInfo(
                            on_wait=keep, on_update=list(si.on_update or [])
                        )
                out.append(ins)
            blk.instructions = out
    for eng in aux_counts:
        # sems persist across NEFF executions; reset so a re-run starts at 0
        nc.gpsimd.sem_clear(range(auxes[eng].num, auxes[eng].num + 1))
    return n


def _weave(*streams, weights=None):
    """Emit thunks from several streams interleaved by fractional progress.
    A stream with weight w emits w times faster than a weight-1 stream, so
    higher-weight streams are front-loaded within the block."""
    pairs = [
        (list(s), (weights[i] if weights else 1.0))
        for i, s in enumerate(streams)
        if s
    ]
    idx = [0] * len(pairs)
    total = sum(len(s) for s, _ in pairs)
    for _ in range(total):
        k = min(
            range(len(pairs)),
            key=lambda i: (idx[i] / (len(pairs[i][0]) * pairs[i][1]), i)
            if idx[i] < len(pairs[i][0])
            else (9.0, i),
        )
        pairs[k][0][idx[k]]()
        idx[k] += 1


def _build_nc():
    nc = bass.Bass("TRN2", num_devices=NCORES)

    xT = nc.dram_tensor("xT", [D, NTOK], BF16, kind="ExternalInput")
    wq = nc.dram_tensor("wq", [D, FEAT], BF16, kind="ExternalInput")
    wk = nc.dram_tensor("wk", [D, FEAT], BF16, kind="ExternalInput")
    wv = nc.dram_tensor("wv", [D, FEAT], BF16, kind="ExternalInput")
    bq = nc.dram_tensor("bq", [FEAT, 1], F32, kind="ExternalInput")
    bk = nc.dram_tensor("bk", [FEAT, 1], F32, kind="ExternalInput")
    bv = nc.dram_tensor("bv", [FEAT, 1], F32, kind="ExternalInput")
    wout = nc.dram_tensor("wout", [D, D], BF16, kind="ExternalInput")
    boutb = nc.dram_tensor("boutb", [1, D], BF16, kind="ExternalInput")
    selg = nc.dram_tensor("selg", [64, 8 * 128], BF16, kind="ExternalInput")
    y = nc.dram_tensor("y", [2 * TOKC, D], F32, kind="ExternalOutput")

    # auxiliary semaphores (one per engine) for the DMA-wait splitting
    # pass; allocated (and cleared) before the TileContext so Tile never
    # recycles their IDs
    aux_sems = {}
    for eng in (mybir.EngineType.SP, mybir.EngineType.Pool,
                mybir.EngineType.PE, mybir.EngineType.DVE,
                mybir.EngineType.Activation):
        aux_sems[eng] = nc.alloc_semaphore(f"wsplit_aux_{eng.name}")
        nc.gpsimd.sem_clear(range(aux_sems[eng].num, aux_sems[eng].num + 1))

    with _SplitDrainTileContext(nc) as tc:
        with (
            tc.tile_pool(name="const", bufs=1) as cp,
            tc.tile_pool(name="work", bufs=3) as wp,
            tc.tile_pool(name="vtp", bufs=2) as vp,
            tc.tile_pool(name="stage", bufs=2) as sp2,
            tc.tile_pool(name="psS", bufs=2, space="PSUM") as psS,
            tc.tile_pool(name="psQ", bufs=2, space="PSUM") as psQ,
            tc.tile_pool(name="psB", bufs=2, space="PSUM") as psB,
            tc.tile_pool(name="dram", bufs=1, space="DRAM") as dp,
        ):
            # ---- persistent SBUF tensors ----
            xt_sb = cp.tile([128, 8 * NTOK], BF16, name="xt_sb")     # 64 KB/part
            wq_sb = cp.tile([128, 8 * FEAT], BF16, name="wq_sb")
            wk_sb = cp.tile([128, 8 * FEAT], BF16, name="wk_sb")
            wv_sb = cp.tile([128, 8 * FEAT], BF16, name="wv_sb")
            bq_sb = cp.tile([FEAT, 1], F32, name="bq_sb")
            bk_sb = cp.tile([FEAT, 1], F32, name="bk_sb")
            bv_sb = cp.tile([FEAT, 1], F32, name="bv_sb")
            bout_sb = cp.tile([1, D], BF16, name="bout_sb")
            mask_sb = cp.tile([128, 128], BF16, name="mask_sb")
            ident_sb = cp.tile([128, 128], BF16, name="ident_sb")
            ones1_sb = cp.tile([1, 128], BF16, name="ones1_sb")
            sel_sb = cp.tile([64, 8 * 128], BF16, name="sel_sb")
            qT_sb = cp.tile([128, NTOK], BF16, name="qT_sb")
            kT_sb = cp.tile([128, NTOK], BF16, name="kT_sb")
            vtok_sb = cp.tile([128, 32 * 130], BF16, name="vtok_sb")
            wout_sb = cp.tile([128, 8 * D], BF16, name="wout_sb")
            ag_sb = [
                cp.tile([128, 8 * TOKC], BF16, name=f"ag_sb{b}") for b in range(2)
            ]
            denb_sb = [
                cp.tile([64, TOKC], BF16, name=f"denb_sb{b}") for b in range(2)
            ]
            denf_sb = [
                cp.tile([64, TOKC], F32, name=f"denf_sb{b}") for b in range(2)
            ]
            denr_sb = [
                cp.tile([64, TOKC], F32, name=f"denr_sb{b}") for b in range(2)
            ]
            denrb_sb = [
                cp.tile([64, TOKC], BF16, name=f"denrb_sb{b}") for b in range(2)
            ]

            # ---- on-chip constants first on the gpsimd stream ----
            make_identity(nc, ident_sb[:])
            # causal triangle for the diagonal 128x128 sub-blocks:
            # mask[p, q] = 1 if q >= p else 0
            nc.gpsimd.memset(mask_sb[:], 1.0)
            nc.gpsimd.affine_select(
                out=mask_sb[:],
                in_=mask_sb[:],
                compare_op=mybir.AluOpType.is_ge,
                fill=0.0,
                base=0,
                pattern=[[1, 128]],
                channel_multiplier=-1,
            )

            # ---- input DMAs (sync HWDGE ring), ordered by first use ----
            def _w_dma(w_sb, wdr):
                for kt in range(8):
                    nc.sync.dma_start(
                        w_sb[:, kt * FEAT : (kt + 1) * FEAT],
                        wdr[kt * 128 : (kt + 1) * 128, :],
                    )

            _w_dma(wk_sb, wk)
            # batch-0 tokens in 512-column pieces, c-major: the first k-proj
            # group (tokens 0:512) is gated on only the first 8 pieces
            for t in range(2):
                for c in range(2):
                    for kt in range(8):
                        base = kt * NTOK + t * 1024 + c * 512
                        nc.sync.dma_start(
                            xt_sb[:, base : base + 512],
                            xT[kt * 128 : (kt + 1) * 128,
                               t * 1024 + c * 512 : t * 1024 + (c + 1) * 512],
                        )
                if t == 0:
                    _w_dma(wq_sb, wq)
            _w_dma(wv_sb, wv)
            for kt in range(8):
                nc.sync.dma_start(
                    xt_sb[:, kt * NTOK + T : kt * NTOK + 2 * T],
                    xT[kt * 128 : (kt + 1) * 128, T : 2 * T],
                )
            for kt in range(8):
                nc.sync.dma_start(
                    wout_sb[:, kt * D : (kt + 1) * D],
                    wout[kt * 128 : (kt + 1) * 128, :],
                )
            nc.gpsimd.dma_start(bk_sb[:], bk[:])
            nc.gpsimd.dma_start(bq_sb[:], bq[:])
            nc.gpsimd.dma_start(bv_sb[:], bv[:])
            nc.gpsimd.dma_start(bout_sb[:], boutb[:])
            nc.gpsimd.dma_start(sel_sb[:], selg[:])

            nc.vector.memset(ones1_sb[:], 1.0)
            vt_view = vtok_sb[:].rearrange("p (g c) -> p g c", c=130)
            nc.vector.memset(vt_view[:, :, 64], 1.0)
            nc.vector.memset(vt_view[:, :, 129], 1.0)
            # selection matrix (host-built): sel[32h+r, r*128 + 64h + i] = 1
            # (i < 64), so sel[:, r*128:(r+1)*128].T @ denrb broadcasts den
            # row 32h+r onto output partitions [64h, 64h+64) for block r.
            # Unused denrb rows must be zero so the broadcast matmul never
            # multiplies 0 * garbage.
            for bb in range(2):
                nc.vector.memset(denrb_sb[bb][:], 0.0)

            # anchors: named instructions set during emission; pending nosync
            # dep edges resolved after ALL emission (so anchors always exist)
            anchors = {}
            pending_deps = []

            def _anchor(key, inst):
                anchors.setdefault(key, inst.ins)

            def _defer_dep(inst, key, why):
                pending_deps.append((inst.ins, key, why))

            # ---- PE keep-warm dummies.  The HAM clock gate only un-throttles
            # after a fully-busy ~3.4us window, and the attention pipeline's
            # natural micro-bubbles (exp -> PSUM-slot coupling) otherwise keep
            # it throttled for the whole kernel.  Operands come from wk (DMA'd
            # first, random data) so the PE datapath actually toggles. ----
            _dummy_phase = [0]

            def dummy_ops(nmm, group=2):
                ops = []
                for i in range(0, nmm, group):
                    def op(n=min(group, nmm - i)):
                        ps_d = psQ.tile([128, 256], F32, tag="q", name="ps_d")
                        for _ in range(n):
                            j = _dummy_phase[0] = (_dummy_phase[0] + 3) % 4
                            nc.tensor.matmul(
                                ps_d[:],
                                wk_sb[:, j * 128 : (j + 1) * 128],
                                wk_sb[:, (j + 2) * 128 : (j + 4) * 128],
                                start=True, stop=True,
                            )
                    ops.append(op)
                return ops

            # ---- QKV projection: dstT[f, tok] = W.T @ x.T (+ bias) ----
            # one group = one [128, 512] PSUM accumulation over 8 k-tiles
            def qkv_group(which, bb, t, c, bias_on_act, vbox=None):
                w_sb, b_sb, dst = {
                    "q": (wq_sb, bq_sb, qT_sb),
                    "k": (wk_sb, bk_sb, kT_sb),
                    "v": (wv_sb, bv_sb, None),
                }[which]
                box = {}

                def mms():
                    if which == "v" and "vt" not in vbox:
                        vbox["vt"] = vp.tile([128, T], BF16, name=f"vT{bb}")
                    box["ps"] = psQ.tile(
                        [128, 512], F32, tag="q", name=f"ps_{which}{bb}{t}{c}"
                    )
                    base = bb * T + t * 1024 + c * 512
                    for kt in range(8):
                        nc.tensor.matmul(
                            box["ps"][:],
                            w_sb[:, kt * FEAT : (kt + 1) * FEAT],
                            xt_sb[:, kt * NTOK + base : kt * NTOK + base + 512],
                            start=(kt == 0),
                            stop=(kt == 7),
                        )

                def bias():
                    off = t * 1024 + c * 512
                    if which == "v":
                        d = vbox["vt"][:, off : off + 512]
                    else:
                        d = dst[:, bb * T + off : bb * T + off + 512]
                    if bias_on_act:
                        nc.scalar.activation(
                            d, box["ps"][:],
                            mybir.ActivationFunctionType.Identity,
                            bias=b_sb[:],
                        )
                    else:
                        nc.vector.tensor_scalar_add(d, box["ps"][:], b_sb[:])

                return [mms, bias]

            # ---- v to token-major (PE transposes), with ones columns ----
            def vpost_ops(bb, vbox):
                ops = []
                for gl in range(16):
                    def op(gl=gl):
                        g = bb * 16 + gl
                        ps_t = psB.tile([128, 128], BF16, tag="pv", name="ps_t")
                        nc.tensor.transpose(
                            ps_t[:], vbox["vt"][:, gl * 128 : (gl + 1) * 128],
                            ident_sb[:],
                        )
                        # single strided copy: cols {0..63} -> vtok[g*130+0..63]
                        # and {64..127} -> vtok[g*130+65..128]
                        dst = vtok_sb[
                            :, g * 130 : g * 130 + 130
                        ].rearrange("p (a b) -> p a b", b=65)[:, :, 0:64]
                        src = ps_t[:].rearrange("p (a b) -> p a b", b=64)
                        nc.vector.tensor_copy(dst, src)
                    ops.append(op)
                return ops

            # ---- attention: h-paired scores + exp + diagonal masks ----
            # scoresT layout: [128 keys, 512 queries] per block; exp on ACT.
            pt_tiles = {}
            a2a_in = {}
            a2a_out = {}
            for bb in range(2):
                for h in range(2):
                    a2a_in[bb, h] = dp.tile(
                        [8, 65, TOKC], BF16, name=f"a2a_in{bb}{h}"
                    )
                    a2a_out[bb, h] = dp.tile(
                        [8, 65, TOKC], BF16, name=f"a2a_out{bb}{h}"
                    )

            def scores_ops(bb, j, hs=(0, 1)):
                """Per kp unit: the (up to two) head-halves' MM pairs emitted
                adjacently so they run concurrently on disjoint row groups,
                then their exps; diagonal-block triangle masks after the last
                two kp units."""
                nk = 4 * (j + 1)
                cb = bb * T
                for h in hs:
                    pt_tiles[h, bb, j] = wp.tile(
                        [128, nk * 512], BF16, tag="pt", name=f"pt_{h}_{bb}_{j}"
                    )
                ops = []
                boxes_store = {}
                for kp in range(nk // 2):
                    def mm4(kp=kp):
                        boxes = {}
                        for h in hs:
                            boxes[h] = psS.tile(
                                [128, 1024], F32, tag="s", name=f"ps2_{h}"
                            )
                        # interleave c within the h pair: h0c0, h1c0, h0c1,
                        # h1c1 -> consecutive MMs sit on different row groups
                        for c in range(2):
                            kk = 2 * kp + c
                            for h in hs:
                                pb = 64 * h
                                inst = nc.tensor.matmul(
                                    boxes[h][:, c * 512 : (c + 1) * 512],
                                    kT_sb[
                                        pb : pb + 64,
                                        cb + kk * 128 : cb + (kk + 1) * 128,
                                    ],
                                    qT_sb[
                                        pb : pb + 64,
                                        cb + j * 512 : cb + (j + 1) * 512,
                                    ],
                                    start=True,
                                    stop=True,
                                )
                                _anchor(("smm", bb, j, h), inst)
                        boxes_store[kp] = boxes

                    def exps(kp=kp):
                        boxes = boxes_store.pop(kp)
                        for h in hs:
                            nc.scalar.activation(
                                pt_tiles[h, bb, j][
                                    :, (2 * kp) * 512 : (2 * kp + 2) * 512
                                ],
                                boxes[h][:],
                                mybir.ActivationFunctionType.Exp,
                                scale=SCALE,
                            )
                    ops.append(mm4)
                    ops.append(exps)
                    if kp >= 2 * j:  # diagonal blocks kk = 4j..4j+3
                        def masks(kp=kp):
                            for h in hs:
                                pt = pt_tiles[h, bb, j]
                                for c in range(2):
                                    kk = 2 * kp + c
                                    r = kk - 4 * j
                                    lo = kk * 512 + 128 * r
                                    inst = nc.vector.tensor_tensor(
                                        pt[:, lo : lo + 128],
                                        pt[:, lo : lo + 128],
                                        mask_sb[:],
                                        mybir.AluOpType.mult,
                                    )
                                    _anchor(("mask", bb, j, h), inst)
                        ops.append(masks)
                return ops

            def pv_ops(h, bb, j):
                nk = 4 * (j + 1)
                pt = pt_tiles.pop((h, bb, j))
                ps_box = {}
                ops = []
                for kk in range(nk):
                    def op(kk=kk, pt=pt):
                        if kk == 0:
                            ps_box["o"] = psB.tile(
                                [65, 512], F32, tag="pv", name="ps_o"
                            )
                        g = bb * 16 + kk
                        r = kk - 4 * j
                        lo = 128 * r if r > 0 else 0
                        inst = nc.tensor.matmul(
                            ps_box["o"][:, lo:512],
                            vtok_sb[:, g * 130 + 65 * h : g * 130 + 65 * h + 65],
                            pt[:, kk * 512 + lo : (kk + 1) * 512],
                            start=(kk == 0),
                            stop=(kk == nk - 1),
                        )
                        _anchor(("pvmm", h, bb, j), inst)
                    ops.append(op)

                def ship():
                    ps_o = ps_box["o"]
                    av = sp2.tile([65, 512], BF16, tag="av", name="av")
                    nc.vector.tensor_copy(av[:], ps_o[:])
                    for t in range(2):
                        nc.sync.dma_start(
                            a2a_in[bb, h][2 * j + t],
                            av[:, t * TOKC : (t + 1) * TOKC],
                        )
                ops.append(ship)
                return ops

            def emit_collective(bb, h):
                nc.gpsimd.collective_compute(
                    "AllToAll",
                    mybir.AluOpType.bypass,
                    replica_groups=[list(range(NCORES))],
                    ins=[a2a_in[bb, h][:]],
                    outs=[a2a_out[bb, h][:]],
                )

            def unpack_ops(bb, h):
                # fast HWDGE ring on the scalar engine; the DMA queue (not
                # the engine) waits on the collective-done semaphore
                def op():
                    nc.scalar.dma_start(
                        denb_sb[bb][32 * h : 32 * h + 8, :],
                        a2a_out[bb, h][:, 64, :],
                    )
                    for r in range(8):
                        nc.scalar.dma_start(
                            ag_sb[bb][
                                64 * h : 64 * h + 64, r * TOKC : (r + 1) * TOKC
                            ],
                            a2a_out[bb, h][r, 0:64, :],
                        )
                return [op]

            # ---- post-A2A normalization + output projection ----
            def recip_ops(bb, h, dep=None):
                def op():
                    sl = slice(32 * h, 32 * h + 8)
                    i1 = nc.vector.tensor_copy(
                        denf_sb[bb][sl, :], denb_sb[bb][sl, :]
                    )
                    if dep is not None:
                        _defer_dep(i1, dep, "hold recip behind stage work")
                    nc.vector.reciprocal(denr_sb[bb][sl, :], denf_sb[bb][sl, :])
                    i3 = nc.vector.tensor_copy(
                        denrb_sb[bb][sl, :], denr_sb[bb][sl, :]
                    )
                    _anchor(("recipd", bb, h), i3)
                return [op]

            def norm_ops(bb, hs=(0, 1), dep=None):
                """Normalize feature rows of the given head-halves, per
                r-block: broadcast-matmul the reciprocal dens then multiply."""
                ops = []
                both = len(hs) == 2
                for r in range(8):
                    def op(r=r):
                        ps_bc = psQ.tile(
                            [128, TOKC], F32, tag="q", name="ps_bc"
                        )
                        if both:
                            mm = nc.tensor.matmul(
                                ps_bc[:],
                                sel_sb[:, r * 128 : (r + 1) * 128],
                                denrb_sb[bb][:, :],
                                start=True, stop=True,
                            )
                            tt = nc.vector.tensor_tensor(
                                ag_sb[bb][:, r * TOKC : (r + 1) * TOKC],
                                ag_sb[bb][:, r * TOKC : (r + 1) * TOKC],
                                ps_bc[:],
                                mybir.AluOpType.mult,
                            )
                        else:
                            h = hs[0]
                            mm = nc.tensor.matmul(
                                ps_bc[64 * h : 64 * h + 64, :],
                                sel_sb[:, r * 128 + 64 * h : r * 128 + 64 * h + 64],
                                denrb_sb[bb][:, :],
                                start=True, stop=True,
                            )
                            tt = nc.vector.tensor_tensor(
                                ag_sb[bb][64 * h : 64 * h + 64,
                                          r * TOKC : (r + 1) * TOKC],
                                ag_sb[bb][64 * h : 64 * h + 64,
                                          r * TOKC : (r + 1) * TOKC],
                                ps_bc[64 * h : 64 * h + 64, :],
                                mybir.AluOpType.mult,
                            )
                        if dep is not None:
                            _defer_dep(mm, dep, "hold norm mm behind stage work")
                            _defer_dep(tt, dep, "hold norm mult behind stage work")
                        _anchor(("ntt", bb, hs), tt)
                    ops.append(op)
                return ops

            def outproj_full_ops(bb, deps=None):
                """Full-contraction output projection (both head halves
                available).  Per m-chunk: two [128, 512] PSUM groups.
                deps: optional per-group anchor keys (m0n0, m0n1, m1n0, m1n1)
                to stagger the groups across late stage work."""
                ops = []
                for m in range(2):
                    boxes = {}
                    for n2 in range(2):
                        def mmgrp(m=m, n2=n2, boxes=boxes):
                            ps = psQ.tile(
                                [128, 512], F32, tag="q", name=f"ps_y{bb}{m}{n2}"
                            )
                            boxes[n2] = ps
                            i0 = nc.tensor.matmul(
                                ps[:],
                                ones1_sb[0:1, 0:128],
                                bout_sb[:, n2 * 512 : (n2 + 1) * 512],
                                start=True,
                                stop=False,
                            )
                            if deps is not None:
                                _defer_dep(i0, deps[2 * m + n2],
                                           "hold outproj behind stage work")
                            for kt in range(8):
                                nc.tensor.matmul(
                                    ps[:],
                                    ag_sb[bb][
                                        :,
                                        kt * TOKC + m * 128 :
                                        kt * TOKC + (m + 1) * 128,
                                    ],
                                    wout_sb[
                                        :,
                                        kt * D + n2 * 512 :
                                        kt * D + (n2 + 1) * 512,
                                    ],
                                    start=False,
                                    stop=(kt == 7),
                                )
                        ops.append(mmgrp)

                    def fin(m=m, boxes=boxes):
                        y_sb = sp2.tile([128, D], F32, tag="ysb", name="y_sb")
                        for n2 in range(2):
                            nc.vector.tensor_copy(
                                y_sb[:, n2 * 512 : (n2 + 1) * 512], boxes[n2][:]
                            )
                        nc.sync.dma_start(
                            y[bb * 256 + m * 128 : bb * 256 + (m + 1) * 128, :],
                            y_sb[:],
                        )
                    ops.append(fin)
                return ops

            def outproj_partial_ops(bb, h, boxes, first, last, dep=None):
                """Half-contraction output projection over feature rows of
                head-half h only.  PSUM groups ([128, 1024] tiles in the
                scores pool, free by now) stay open between the two halves
                so the first half runs while the last A2A is in flight."""
                ops = []
                for m in range(2):
                    box = boxes[m]
                    def grp(m=m, box=box):
                        if first:
                            box["ps"] = psS.tile(
                                [128, 1024], F32, tag="s", name=f"ps_yp{m}"
                            )
                        ps = box["ps"]
                        for n2 in range(2):
                            if first:
                                i0 = nc.tensor.matmul(
                                    ps[:, n2 * 512 : (n2 + 1) * 512],
                                    ones1_sb[0:1, 0:128],
                                    bout_sb[:, n2 * 512 : (n2 + 1) * 512],
                                    start=True,
                                    stop=False,
                                )
                                if dep is not None:
                                    _defer_dep(
                                        i0, dep,
                                        "hold partial outproj behind stage work",
                                    )
                            for kt in range(8):
                                nc.tensor.matmul(
                                    ps[:, n2 * 512 : (n2 + 1) * 512],
                                    ag_sb[bb][
                                        64 * h : 64 * h + 64,
                                        kt * TOKC + m * 128 :
                                        kt * TOKC + (m + 1) * 128,
                                    ],
                                    wout_sb[
                                        64 * h : 64 * h + 64,
                                        kt * D + n2 * 512 :
                                        kt * D + (n2 + 1) * 512,
                                    ],
                                    start=False,
                                    stop=(last and kt == 7),
                                )
                    ops.append(grp)
                    if last:
                        def fin(m=m, box=box):
                            y_sb = sp2.tile(
                                [128, D], F32, tag="ysb", name="y_sb"
                            )
                            nc.vector.tensor_copy(y_sb[:], box["ps"][:])
                            nc.sync.dma_start(
                                y[bb * 256 + m * 128 :
                                  bb * 256 + (m + 1) * 128, :],
                                y_sb[:],
                            )
                        ops.append(fin)
                return ops

            # ================= emission schedule =================
            # --- prologue: warm-up + batch-0 K/Q (t=0 first) ---
            kq_t0 = (qkv_group("k", 0, 0, 0, True) + qkv_group("k", 0, 0, 1, True)
                     + qkv_group("q", 0, 0, 0, True) + qkv_group("q", 0, 0, 1, True))
            _weave(dummy_ops(24), kq_t0, weights=(1.0, 1.2))

            # --- phase A: batch-0 attention, woven with the rest of QKV ---
            vbox0, vbox1 = {}, {}
            fillA = []
            for t, c in ((0, 0), (0, 1)):
                fillA += qkv_group("v", 0, t, c, False, vbox0)
            fillA += vpost_ops(0, vbox0)[:8]
            for wch in ("k", "q"):
                for c in range(2):
                    fillA += qkv_group(wch, 0, 1, c, True)
            for t, c in ((1, 0), (1, 1)):
                fillA += qkv_group("v", 0, t, c, False, vbox0)
            fillA += vpost_ops(0, vbox0)[8:]
            for wch in ("k", "q", "v"):
                for t in range(2):
                    for c in range(2):
                        fillA += qkv_group(
                            wch, 1, t, c, False,
                            vbox1 if wch == "v" else None,
                        )
            fillA += vpost_ops(1, vbox1)

            nfa = len(fillA)
            # fill chunks front-loaded so vtok b0 is ready before the PV
            # that needs it
            cuts = [0, int(nfa * 0.22), int(nfa * 0.42), int(nfa * 0.68), nfa]
            fillA_chunks = [fillA[cuts[i]: cuts[i + 1]] for i in range(4)]

            prev = None
            for j in range(4):
                a = scores_ops(0, j)
                b = (pv_ops(0, 0, prev) + pv_ops(1, 0, prev)) if prev is not None else []
                _weave(a, b, fillA_chunks[j], dummy_ops(8),
                       weights=(1.0, 1.0, 1.4 if j == 0 else 1.0, 0.9))
                prev = j
            _weave(pv_ops(0, 0, 3), dummy_ops(6))
            emit_collective(0, 0)
            _weave(pv_ops(1, 0, 3), dummy_ops(6))
            emit_collective(0, 1)

            # --- phase B: batch-1 attention, woven with batch-0 post ---
            for op in unpack_ops(0, 0) + unpack_ops(0, 1):
                op()
            # outproj(0) groups get staggered anchors so the scheduler
            # interleaves them with the late-phase-B stages instead of
            # packing them into one ACT-starving block
            op0 = outproj_full_ops(0, deps=[("smm", 1, 2, 0), ("smm", 1, 2, 1),
                                            ("smm", 1, 3, 1), ("smm", 1, 3, 0)])
            postB = (recip_ops(0, 0, dep=("mask", 1, 1, 0))
                     + recip_ops(0, 1, dep=("mask", 1, 1, 0))
                     + norm_ops(0, dep=("smm", 1, 2, 0)))
            npb = len(postB)
            bcuts = [0, 0, int(npb * 0.5), npb]
            postB_chunks = [postB[bcuts[i]: bcuts[i + 1]] for i in range(3)]

            prev = None
            for j in range(3):
                a = scores_ops(1, j)
                b = (pv_ops(0, 1, prev) + pv_ops(1, 1, prev)) if prev is not None else []
                _weave(a, b, postB_chunks[j], dummy_ops(8), weights=(1, 1, 1, 0.9))
                prev = j

            # j=3 split: h1 first so its A2A fires ~one solo-stage early.
            # outproj(0) m=0 groups interleave here (anchored at j=2 stages).
            a_h1 = scores_ops(1, 3, hs=(1,))
            b = pv_ops(0, 1, 2) + pv_ops(1, 1, 2)
            _weave(a_h1, b, op0[0:3], dummy_ops(6))
            for op in pv_ops(1, 1, 3):
                op()
            emit_collective(1, 1)
            for op in unpack_ops(1, 1):
                op()

            # h0's final stage runs while A2A(1,1) is in flight; outproj(0)
            # m=1 groups and the h1 tail norm weave through it
            a_h0 = scores_ops(1, 3, hs=(0,))
            tail1 = (op0[3:6]
                     + recip_ops(1, 1, dep=("smm", 1, 3, 0))
                     + norm_ops(1, hs=(1,), dep=("smm", 1, 3, 0)))
            _weave(a_h0, tail1, dummy_ops(10), weights=(1.0, 0.8, 0.7))
            for op in pv_ops(0, 1, 3):
                op()
            emit_collective(1, 0)
            for op in unpack_ops(1, 0):
                op()

            # h1-half output projection overlaps A2A(1,0); h0 half lands after
            boxes1 = [{} for _ in range(2)]
            _weave(outproj_partial_ops(1, 1, boxes1, True, False,
                                       dep=("pvmm", 0, 1, 3)),
                   dummy_ops(10), weights=(1.0, 0.6))
            _weave(recip_ops(1, 0, dep=("ntt", 1, (1,)))
                   + norm_ops(1, hs=(0,))
                   + outproj_partial_ops(1, 0, boxes1, False, True),
                   dummy_ops(12), weights=(1.0, 0.5))

            # resolve deferred nosync ordering edges now that all anchors exist
            for ins, key, why in pending_deps:
                if key in anchors:
                    add_dep_helper(ins, anchors[key], False, why)

    _split_excess_waits(nc, aux_sems)
    return nc


_NC_CACHE = None


def _get_nc():
    global _NC_CACHE
    if _NC_CACHE is None:
        _NC_CACHE = _build_nc()
    return _NC_CACHE


def kernel(x, Wqkv, bqkv, Wout, bout):
    global LAST_RESULT
    x = np.asarray(x, dtype=np.float32)
    Wqkv = np.asarray(Wqkv, dtype=np.float32)
    bqkv = np.asarray(bqkv, dtype=np.float32)
    Wout = np.asarray(Wout, dtype=np.float32)
    bout = np.asarray(bout, dtype=np.float32)

    Bx, Tx, Dx = x.shape
    assert (Bx, Tx, Dx) == (B, T, D)

    xT = np.ascontiguousarray(x.reshape(NTOK, D).T).astype(NPBF16)
    wq_full = Wqkv[:, 0:D]
    wk_full = Wqkv[:, D : 2 * D]
    wv_full = Wqkv[:, 2 * D : 3 * D]
    bq_full = bqkv[0:D]
    bk_full = bqkv[D : 2 * D]
    bv_full = bqkv[2 * D : 3 * D]

    wout_b = np.ascontiguousarray(Wout).astype(NPBF16)
    boutb = np.ascontiguousarray(bout.reshape(1, D)).astype(NPBF16)
    selg = np.zeros((64, 8 * 128), dtype=NPBF16)
    for r in range(8):
        for h in range(2):
            selg[32 * h + r, r * 128 + 64 * h : r * 128 + 64 * h + 64] = 1

    in_maps = []
    for c in range(NCORES):
        sl = slice(FEAT * c, FEAT * (c + 1))
        in_maps.append(
            {
                "xT": xT,
                "wq": np.ascontiguousarray(wq_full[:, sl]).astype(NPBF16),
                "wk": np.ascontiguousarray(wk_full[:, sl]).astype(NPBF16),
                "wv": np.ascontiguousarray(wv_full[:, sl]).astype(NPBF16),
                "bq": np.ascontiguousarray(bq_full[sl].reshape(FEAT, 1)),
                "bk": np.ascontiguousarray(bk_full[sl].reshape(FEAT, 1)),
                "bv": np.ascontiguousarray(bv_full[sl].reshape(FEAT, 1)),
                "wout": wout_b,
                "boutb": boutb,
                "selg": selg,
            }
        )

    nc = _get_nc()
    res = run_bass_kernel_spmd(
        nc,
        in_maps,
        core_ids=list(range(NCORES)),
        trace=TRACE,
        **TRACE_KWARGS,
    )
    LAST_RESULT = res
    out = np.empty((B, T, D), dtype=np.float32)
    for c in range(NCORES):
        yc = res.results[c]["y"]
        out[0, c * TOKC : (c + 1) * TOKC, :] = yc[0:TOKC]
        out[1, c * TOKC : (c + 1) * TOKC, :] = yc[TOKC : 2 * TOKC]
    return out
